# revision 27
# baseline (speedup 1.0000x reference)
"""Trainium2 Bass kernel: GQA sliding-window attention.

Problem: B=1, T=4096, D=2048, H=16 q-heads, KVH=4 kv-heads, HD=128,
causal sliding window 512.

Sharding: 8-way sequence parallel. Core c owns query rows
[512c, 512c+512). It receives x rows [512(c-1), 512(c+1)) (halo of 512
rows for the attention window; core 0's halo is zeros and is masked
out). Weights are replicated. Outputs are disjoint row blocks -> plain
concatenation, no collectives.

Per-core layouts (SBUF partition dim first):
  xT  [128, 16, 1024] : xT[p, dc, t] = x[t, 128*dc+p]
  qT  [128, 16, 512]  : qT[p, h, i]  = q[i, 128*h+p] * SCALE  (own rows)
  kT  [128, 4, 1024]  : kT[p, g, j]  = k[j, 128*g+p]
  vv  [128, 8, 512]   : vv[p, jc, e] = v[128*jc+p, e]  (e = 128*g+hd)
  Scores per (head h, q-tile t): s[i', jj], key j = 128*t + jj,
    jj in [0,640). Additive host mask allows jj in [i', i'+512]
    (plus key >= first real row on core 0).
  Softmax without max-subtraction (scores bounded for this input
    distribution; verified host-side in the test harness).
  Normalization is folded into the PE transpose of the attention
    weights: transpose rhs = diag(1/rowsum) instead of identity.
  wT [128, 8, 512] holds transposed weights; out-of-band blocks
    zeroed once.
  oT  [128, 16, 512]  : oT[p, h, i] = attn_out[i, 128*h+p]
  y = oT.T @ Wo accumulated over heads, Wo streamed in 512-col blocks.
"""

import numpy as np

T = 4096
D = 2048
H = 16
KVH = 4
HD = 128
WINDOW = 512
SCALE = HD ** -0.5
N_CORES = 8
TLOC = T // N_CORES          # 512 own query rows / core
XROWS = TLOC + WINDOW        # 1024 x rows / core (halo + own)
NT = TLOC // 128             # 4 q-tiles of 128 rows
NJC = XROWS // 128           # 8 key chunks of 128
BAND = WINDOW + 128          # 640 key columns per q-tile
DC = D // 128                # 16 d-chunks
MASK_VAL = -1e10

_CACHE = {}


def _emit(nc, tc, tile, mybir, make_identity, loop_n=None, stop_after=None):
    f32 = mybir.dt.float32
    f32r = mybir.dt.float32r

    timing = loop_n is not None
    kin = "Internal" if timing else "ExternalInput"
    kout = "Internal" if timing else "ExternalOutput"
    x_d = nc.dram_tensor("x", [XROWS, D], f32, kind=kin)
    wq_d = nc.dram_tensor("wq", [D, H * HD], f32, kind=kin)
    wk_d = nc.dram_tensor("wk", [D, KVH * HD], f32, kind=kin)
    wv_d = nc.dram_tensor("wv", [D, KVH * HD], f32, kind=kin)
    wo_d = nc.dram_tensor("wo", [H * HD, D], f32, kind=kin)
    mask_d = nc.dram_tensor("mask", [NT, 128, BAND], f32, kind=kin)
    y_d = nc.dram_tensor("y", [TLOC, D], f32, kind=kout)
    if timing:
        dummy_d = nc.dram_tensor("bench_done", [1, 128], f32,
                                 kind="ExternalOutput")

    def mm(out, lhsT, rhs, start, stop):
        nc.tensor.matmul(out, lhsT, rhs, start=start, stop=stop)

    # --- long-lived pools ---
    # PSUM budget (8 banks): ps_s (tags sA+sB, 2 bufs each) = 4,
    # ps_ot = 2, plus one phase-scoped right-side pool of 2.
    pers = tc.alloc_tile_pool(name="pers", bufs=1)
    ps_s = tc.alloc_tile_pool(name="ps_s", bufs=2, space="PSUM")
    ps_ot = tc.alloc_tile_pool(name="ps_ot", bufs=2, space="PSUM")

    ident = pers.tile([128, 128], f32, tag="ident")
    make_identity(nc, ident[:])
    identr = pers.tile([128, 128], f32r, tag="identr")
    nc.vector.tensor_copy(identr[:], ident[:])

    lp = tc.For_i(0, loop_n, 1) if timing else None
    if lp is not None:
        lp.__enter__()

    proj = tc.alloc_tile_pool(name="proj", bufs=1)
    ps_acc = tc.alloc_tile_pool(name="ps_acc", bufs=2, space="PSUM",
                                side="right")
    mask_s = proj.tile([128, NT, BAND], f32, tag="mask")
    qT = proj.tile([128, H, TLOC], f32r, tag="qT")
    kT = proj.tile([128, KVH, XROWS], f32r, tag="kT")
    vv = proj.tile([128, NJC, KVH * HD], f32r, tag="vv")

    # ---------------- P0: load x, build xT ----------------
    # xT split: halo rows [0,512) in xTa (right side, freed after k/v),
    # own rows [512,1024) in xTb (lives until q projections are done).
    p0b = tc.alloc_tile_pool(name="p0b", bufs=1)
    wp = tc.alloc_tile_pool(name="wpool", bufs=2)
    xl = tc.alloc_tile_pool(name="xload", bufs=2)
    p0a = tc.alloc_tile_pool(name="p0a", bufs=1, side="right")
    xTa = p0a.tile([128, DC, WINDOW], f32r, tag="xTa")
    xTb = p0b.tile([128, DC, WINDOW], f32r, tag="xTb")

    def xTc(dc, lo, hi):
        """xT[:, dc, lo:hi] across the a/b split (lo, hi within a half)."""
        if hi <= WINDOW:
            return xTa[:, dc, lo:hi]
        assert lo >= WINDOW
        return xTb[:, dc, lo - WINDOW:hi - WINDOW]

    for r in range(NJC):
        xr = xl.tile([128, D], f32r, tag="xload")
        nc.sync.dma_start(xr[:],
                          x_d.ap()[r * 128:(r + 1) * 128, :].bitcast(f32r))
        xt_half = xTa if r < 4 else xTb
        rr = r % 4
        for dcg in range(4):
            pt = ps_s.tile([128, 512], f32r, tag="score")
            for i in range(4):
                dc = dcg * 4 + i
                nc.tensor.transpose(
                    pt[:, i * 128:(i + 1) * 128],
                    xr[:, dc * 128:(dc + 1) * 128],
                    identr[:])
            nc.vector.tensor_copy(
                xt_half[:, dcg * 4:(dcg + 1) * 4, rr * 128:(rr + 1) * 128],
                pt[:])

    # mask load (needed only in P2; emitted after x so x wins the queue)
    nc.sync.dma_start(mask_s[:], mask_d.ap().rearrange("t p j -> p t j"))

    # ---------------- P1a: k/v projections ----------------
    for g in range(KVH):
        wkg = wp.tile([128, DC, 128], f32r, tag="wlhs")
        nc.sync.dma_start(
            wkg[:],
            wk_d.ap()[:, g * 128:(g + 1) * 128]
            .rearrange("(c p) e -> p c e", p=128).bitcast(f32r))
        for half in range(2):
            pk = ps_acc.tile([128, 512], f32, tag="acc")
            for dc in range(DC):
                mm(pk[:], wkg[:, dc, :],
                   xTc(dc, half * 512, (half + 1) * 512),
                   start=(dc == 0), stop=(dc == DC - 1))
            nc.vector.tensor_copy(kT[:, g, half * 512:(half + 1) * 512], pk[:])

    # v chunks in two 256-wide e halves: vv[p, jc, e] = v[128*jc+p, e]
    wvp = tc.alloc_tile_pool(name="wvp", bufs=2)
    for half in range(2):
        wvt = wvp.tile([128, DC, 256], f32r, tag="wvh")
        nc.sync.dma_start(
            wvt[:],
            wv_d.ap()[:, half * 256:(half + 1) * 256]
            .rearrange("(c p) e -> p c e", p=128).bitcast(f32r))
        for jc in range(NJC):
            pv = ps_acc.tile([128, 256], f32, tag="acc")
            for dc in range(DC):
                mm(pv[:], xTc(dc, jc * 128, (jc + 1) * 128), wvt[:, dc, :],
                   start=(dc == 0), stop=(dc == DC - 1))
            nc.vector.tensor_copy(vv[:, jc, half * 256:(half + 1) * 256], pv[:])

    wvp.release()
    xl.release()
    p0a.release()
    ps_acc.release()

    if stop_after == "kv":
        if lp is not None:
            lp.__exit__(None, None, None)
            dtile = pers.tile([128, 128], f32, tag="dtile")
            nc.vector.memset(dtile[:], 0.0)
            nc.sync.dma_start(dummy_d.ap(), dtile[0:1, :])
        wp.release()
        p0b.release()
        proj.release()
        ps_ot.release()
        ps_s.release()
        pers.release()
        return

    # ---------------- P2: attention, q projection interleaved ----------
    attn = tc.alloc_tile_pool(name="attn", bufs=1, side="right")
    sm = tc.alloc_tile_pool(name="sm", bufs=2, side="right")
    ps_wtp = tc.alloc_tile_pool(name="ps_wtp", bufs=2, space="PSUM",
                                side="right")

    oT = attn.tile([128, H, TLOC], f32r, tag="oT")
    wT = attn.tile([128, NJC, TLOC], f32r, tag="wT")
    # zero the out-of-band blocks of wT once (via f32 scratch; gpsimd
    # memset cannot encode f32r)
    zblk = attn.tile([128, 384], f32, tag="zblk")
    nc.gpsimd.memset(zblk[:], 0.0)
    for jc, (a, b) in enumerate(
            [(128, 512), (256, 512), (384, 512), (None, None),
             (None, None), (0, 128), (0, 256), (0, 384)]):
        if a is not None:
            nc.vector.tensor_copy(wT[:, jc, a:b], zblk[:, 0:b - a])

    def stage_a1(h):
        """q projection + scores/softmax for q-tiles 0,1 of head h."""
        g = h // (H // KVH)
        wqh = wp.tile([128, DC, 128], f32r, tag="wlhs", name=f"wqh{h}")
        nc.sync.dma_start(
            wqh[:],
            wq_d.ap()[:, h * 128:(h + 1) * 128]
            .rearrange("(c p) e -> p c e", p=128).bitcast(f32r))
        pq = ps_ot.tile([128, TLOC], f32, tag="ot", name=f"pq{h}")
        for dc in range(DC):
            mm(pq[:], wqh[:, dc, :], xTb[:, dc, :],
               start=(dc == 0), stop=(dc == DC - 1))
        nc.scalar.mul(qT[:, h, :], pq[:], SCALE)
        return [softmax_tile(h, g, t) for t in (0, 1)]

    def stage_a2(h, wt_list):
        g = h // (H // KVH)
        wt_list += [softmax_tile(h, g, t) for t in (2, 3)]
        return wt_list

    def softmax_tile(h, g, t):
        ps = ps_s.tile([128, 1024], f32, tag="score", name=f"s{h}_{t}")
        mm(ps[:, 0:512], qT[:, h, t * 128:(t + 1) * 128],
           kT[:, g, t * 128:t * 128 + 512], start=True, stop=True)
        mm(ps[:, 512:BAND], qT[:, h, t * 128:(t + 1) * 128],
           kT[:, g, t * 128 + 512:t * 128 + BAND], start=True, stop=True)
        w = sm.tile([128, BAND], f32r, tag="w", name=f"w{h}_{t}", bufs=10)
        nc.scalar.activation(w[:], ps[:, 0:BAND],
                             mybir.ActivationFunctionType.Exp)
        lsum = sm.tile([128, 1], f32, tag="l", name=f"l{h}_{t}", bufs=4)
        # multiplicative mask + row-sum in one DVE pass (in place)
        nc.vector.scalar_tensor_tensor(
            w[:], w[:], 1.0, mask_s[:, t, :],
            op0=mybir.AluOpType.mult, op1=mybir.AluOpType.mult,
            accum_out=lsum[:])
        r = sm.tile([128, 1], f32, tag="r", name=f"r{h}_{t}", bufs=4)
        nc.vector.reciprocal(r[:], lsum[:])
        if t < 2:
            nc.vector.tensor_scalar_mul(w[:], w[:], r[:])
        else:
            nc.scalar.mul(w[:], w[:], r[:])
        return w

    def stage_b(h, wt_list):
        """transpose + PV + oT for head h."""
        g = h // (H // KVH)
        for jc in range(NJC):
            t_lo = max(0, jc - 4)
            t_hi = min(NT - 1, jc)
            pt = ps_wtp.tile([128, 512], f32r, tag="wtp",
                             name=f"pt{h}_{jc}")
            for t in range(t_lo, t_hi + 1):
                co = jc - t  # w column block
                nc.tensor.transpose(
                    pt[:, t * 128:(t + 1) * 128],
                    wt_list[t][:, co * 128:(co + 1) * 128],
                    identr[:])
            if jc % 2 == 0:
                nc.vector.tensor_copy(
                    wT[:, jc, t_lo * 128:(t_hi + 1) * 128],
                    pt[:, t_lo * 128:(t_hi + 1) * 128])
            else:
                nc.scalar.copy(
                    wT[:, jc, t_lo * 128:(t_hi + 1) * 128],
                    pt[:, t_lo * 128:(t_hi + 1) * 128])

        po = ps_ot.tile([128, TLOC], f32, tag="ot", name=f"po{h}")
        for jc in range(NJC):
            mm(po[:], vv[:, jc, g * 128:(g + 1) * 128], wT[:, jc, :],
               start=(jc == 0), stop=(jc == NJC - 1))
        nc.vector.tensor_copy(oT[:, h, :], po[:])

    # 2-stage software pipeline: head h's q/scores are emitted before
    # head h-1's transposes/PV so the PE queue always has work while
    # DVE/ACT run the softmax chain; scores t2,t3 go after stage_b so
    # their PSUM slots (freed by exp) are ready when the PE gets there.
    prev = None
    for h in range(H):
        cur = stage_a1(h)
        if prev is not None:
            stage_b(h - 1, prev)
        cur = stage_a2(h, cur)
        prev = cur
    stage_b(H - 1, prev)

    sm.release()
    ps_wtp.release()
    wp.release()
    p0b.release()
    proj.release()

    if stop_after == "attn":
        if lp is not None:
            lp.__exit__(None, None, None)
            dtile = pers.tile([128, 128], f32, tag="dtile")
            nc.vector.memset(dtile[:], 0.0)
            nc.sync.dma_start(dummy_d.ap(), dtile[0:1, :])
        attn.release()
        ps_ot.release()
        ps_s.release()
        pers.release()
        return

    # ---------------- P3: output projection ----------------
    wop = tc.alloc_tile_pool(name="wo_pool", bufs=3, side="right")
    ps_acc2 = tc.alloc_tile_pool(name="ps_acc2", bufs=2, space="PSUM",
                                 side="right")
    for dblk in range(4):
        woc = wop.tile([128, H, 512], f32r, tag="wo")
        nc.sync.dma_start(
            woc[:],
            wo_d.ap()[:, dblk * 512:(dblk + 1) * 512]
            .rearrange("(h p) e -> p h e", p=128).bitcast(f32r))
        for t in range(NT):
            py = ps_acc2.tile([128, 512], f32, tag="acc")
            for h in range(H):
                mm(py[:], oT[:, h, t * 128:(t + 1) * 128], woc[:, h, :],
                   start=(h == 0), stop=(h == H - 1))
            ych = attn.tile([128, 512], f32, tag="ych", bufs=2)
            nc.vector.tensor_copy(ych[:], py[:])
            nc.scalar.dma_start(
                y_d.ap()[t * 128:(t + 1) * 128,
                         dblk * 512:(dblk + 1) * 512],
                ych[:])

    ps_acc2.release()
    wop.release()
    attn.release()

    if lp is not None:
        lp.__exit__(None, None, None)
        dtile = pers.tile([128, 128], f32, tag="dtile")
        nc.vector.memset(dtile[:], 0.0)
        nc.sync.dma_start(dummy_d.ap(), dtile[0:1, :])

    ps_ot.release()
    ps_s.release()
    pers.release()


def build_nc(loop_n=None, stop_after=None):
    key = ("nc", loop_n, stop_after)
    if key in _CACHE:
        return _CACHE[key]
    import concourse.bacc as bacc
    import concourse.mybir as mybir
    import concourse.tile as tile
    from concourse.masks import make_identity

    nc = bacc.Bacc("TRN2", target_bir_lowering=False, debug=False,
                   num_devices=N_CORES)
    with tile.TileContext(nc) as tc:
        _emit(nc, tc, tile, mybir, make_identity, loop_n=loop_n,
          stop_after=stop_after)
    nc.compile()
    _CACHE[key] = nc
    return nc


def make_inputs_for_core(c, xf, Wq, Wk, Wv, Wo):
    """xf: [T, D] float32 (already squeezed)."""
    if c == 0:
        x_c = np.concatenate(
            [np.zeros((WINDOW, D), np.float32), xf[:TLOC]], axis=0)
    else:
        x_c = xf[TLOC * c - WINDOW: TLOC * c + TLOC]
    x_c = np.ascontiguousarray(x_c, dtype=np.float32)

    jj = np.arange(BAND)[None, None, :]
    p = np.arange(128)[None, :, None]
    t = np.arange(NT)[:, None, None]
    allowed = (jj >= p) & (jj <= p + WINDOW)
    if c == 0:
        allowed = allowed & (128 * t + jj >= WINDOW)
    allowed = np.broadcast_to(allowed, (NT, 128, BAND))
    mask = np.where(allowed, np.float32(1.0),
                    np.float32(0.0)).astype(np.float32)

    return {
        "x": x_c,
        "wq": np.ascontiguousarray(Wq, np.float32),
        "wk": np.ascontiguousarray(Wk, np.float32),
        "wv": np.ascontiguousarray(Wv, np.float32),
        "wo": np.ascontiguousarray(Wo, np.float32),
        "mask": mask,
    }


def kernel(x, Wq, Wk, Wv, Wo):
    from concourse.bass_utils import run_bass_kernel_spmd

    nc = build_nc()
    xf = np.asarray(x, np.float32).reshape(T, D)
    in_maps = [make_inputs_for_core(c, xf, Wq, Wk, Wv, Wo)
               for c in range(N_CORES)]
    res = run_bass_kernel_spmd(nc, in_maps, core_ids=list(range(N_CORES)))
    y = np.concatenate([res.results[c]["y"] for c in range(N_CORES)], axis=0)
    return y.reshape(1, T, D)


# revision 29
# speedup vs baseline: 1.0442x; 1.0442x over previous
"""Trainium2 Bass kernel: GQA sliding-window attention.

Problem: B=1, T=4096, D=2048, H=16 q-heads, KVH=4 kv-heads, HD=128,
causal sliding window 512.

Sharding: 8-way sequence parallel. Core c owns query rows
[512c, 512c+512). It receives x rows [512(c-1), 512(c+1)) (halo of 512
rows for the attention window; core 0's halo is zeros and is masked
out). Weights are replicated. Outputs are disjoint row blocks -> plain
concatenation, no collectives.

Per-core layouts (SBUF partition dim first):
  xT  [128, 16, 1024] : xT[p, dc, t] = x[t, 128*dc+p]
  qT  [128, 16, 512]  : qT[p, h, i]  = q[i, 128*h+p] * SCALE  (own rows)
  kT  [128, 4, 1024]  : kT[p, g, j]  = k[j, 128*g+p]
  vv  [128, 8, 512]   : vv[p, jc, e] = v[128*jc+p, e]  (e = 128*g+hd)
  Scores per (head h, q-tile t): s[i', jj], key j = 128*t + jj,
    jj in [0,640). Additive host mask allows jj in [i', i'+512]
    (plus key >= first real row on core 0).
  Softmax without max-subtraction (scores bounded for this input
    distribution; verified host-side in the test harness).
  Normalization is folded into the PE transpose of the attention
    weights: transpose rhs = diag(1/rowsum) instead of identity.
  wT [128, 8, 512] holds transposed weights; out-of-band blocks
    zeroed once.
  oT  [128, 16, 512]  : oT[p, h, i] = attn_out[i, 128*h+p]
  y = oT.T @ Wo accumulated over heads, Wo streamed in 512-col blocks.
"""

import numpy as np

T = 4096
D = 2048
H = 16
KVH = 4
HD = 128
WINDOW = 512
SCALE = HD ** -0.5
N_CORES = 8
TLOC = T // N_CORES          # 512 own query rows / core
XROWS = TLOC + WINDOW        # 1024 x rows / core (halo + own)
NT = TLOC // 128             # 4 q-tiles of 128 rows
NJC = XROWS // 128           # 8 key chunks of 128
BAND = WINDOW + 128          # 640 key columns per q-tile
DC = D // 128                # 16 d-chunks
MASK_VAL = -1e10

_CACHE = {}


def _emit(nc, tc, tile, mybir, make_identity, loop_n=None, stop_after=None):
    f32 = mybir.dt.float32
    f32r = mybir.dt.float32r

    timing = loop_n is not None
    kin = "Internal" if timing else "ExternalInput"
    kout = "Internal" if timing else "ExternalOutput"
    x_d = nc.dram_tensor("x", [XROWS, D], f32, kind=kin)
    wq_d = nc.dram_tensor("wq", [D, H * HD], f32, kind=kin)
    wk_d = nc.dram_tensor("wk", [D, KVH * HD], f32, kind=kin)
    wv_d = nc.dram_tensor("wv", [D, KVH * HD], f32, kind=kin)
    wo_d = nc.dram_tensor("wo", [H * HD, D], f32, kind=kin)
    mask_d = nc.dram_tensor("mask", [NT, 128, BAND], f32, kind=kin)
    y_d = nc.dram_tensor("y", [TLOC, D], f32, kind=kout)
    if timing:
        dummy_d = nc.dram_tensor("bench_done", [1, 128], f32,
                                 kind="ExternalOutput")

    def mm(out, lhsT, rhs, start, stop):
        nc.tensor.matmul(out, lhsT, rhs, start=start, stop=stop)

    # --- long-lived pools ---
    # PSUM budget (8 banks): ps_s (tags sA+sB, 2 bufs each) = 4,
    # ps_ot = 2, plus one phase-scoped right-side pool of 2.
    pers = tc.alloc_tile_pool(name="pers", bufs=1)
    ps_s = tc.alloc_tile_pool(name="ps_s", bufs=2, space="PSUM")
    ps_ot = tc.alloc_tile_pool(name="ps_ot", bufs=2, space="PSUM")

    ident = pers.tile([128, 128], f32, tag="ident")
    make_identity(nc, ident[:])
    identr = pers.tile([128, 128], f32r, tag="identr")
    nc.vector.tensor_copy(identr[:], ident[:])

    lp = tc.For_i(0, loop_n, 1) if timing else None
    if lp is not None:
        lp.__enter__()

    proj = tc.alloc_tile_pool(name="proj", bufs=1)
    ps_acc = tc.alloc_tile_pool(name="ps_acc", bufs=2, space="PSUM",
                                side="right")
    mask_s = proj.tile([128, NT, BAND], f32, tag="mask")
    qT = proj.tile([128, H, TLOC], f32r, tag="qT")
    kT = proj.tile([128, KVH, XROWS], f32r, tag="kT")
    vv = proj.tile([128, NJC, KVH * HD], f32r, tag="vv")

    # ---------------- P0: load x, build xT ----------------
    # xT split: halo rows [0,512) in xTa (right side, freed after k/v),
    # own rows [512,1024) in xTb (lives until q projections are done).
    p0b = tc.alloc_tile_pool(name="p0b", bufs=1)
    wp = tc.alloc_tile_pool(name="wpool", bufs=2)
    xl = tc.alloc_tile_pool(name="xload", bufs=2)
    p0a = tc.alloc_tile_pool(name="p0a", bufs=1, side="right")
    xTa = p0a.tile([128, DC, WINDOW], f32r, tag="xTa")
    xTb = p0b.tile([128, DC, WINDOW], f32r, tag="xTb")

    def xTc(dc, lo, hi):
        """xT[:, dc, lo:hi] across the a/b split (lo, hi within a half)."""
        if hi <= WINDOW:
            return xTa[:, dc, lo:hi]
        assert lo >= WINDOW
        return xTb[:, dc, lo - WINDOW:hi - WINDOW]

    for r in range(NJC):
        xr = xl.tile([128, D], f32r, tag="xload")
        nc.sync.dma_start(xr[:],
                          x_d.ap()[r * 128:(r + 1) * 128, :].bitcast(f32r))
        xt_half = xTa if r < 4 else xTb
        rr = r % 4
        for dcg in range(4):
            pt = ps_s.tile([128, 512], f32r, tag="score")
            for i in range(4):
                dc = dcg * 4 + i
                nc.tensor.transpose(
                    pt[:, i * 128:(i + 1) * 128],
                    xr[:, dc * 128:(dc + 1) * 128],
                    identr[:])
            nc.vector.tensor_copy(
                xt_half[:, dcg * 4:(dcg + 1) * 4, rr * 128:(rr + 1) * 128],
                pt[:])

    # mask load (needed only in P2; emitted after x so x wins the queue)
    nc.sync.dma_start(mask_s[:], mask_d.ap().rearrange("t p j -> p t j"))

    # ---------------- P1a: k/v projections ----------------
    for g in range(KVH):
        wkg = wp.tile([128, DC, 128], f32r, tag="wlhs")
        nc.sync.dma_start(
            wkg[:],
            wk_d.ap()[:, g * 128:(g + 1) * 128]
            .rearrange("(c p) e -> p c e", p=128).bitcast(f32r))
        for half in range(2):
            pk = ps_acc.tile([128, 512], f32, tag="acc")
            for dc in range(DC):
                mm(pk[:], wkg[:, dc, :],
                   xTc(dc, half * 512, (half + 1) * 512),
                   start=(dc == 0), stop=(dc == DC - 1))
            nc.vector.tensor_copy(kT[:, g, half * 512:(half + 1) * 512], pk[:])

    # v chunks in two 256-wide e halves: vv[p, jc, e] = v[128*jc+p, e]
    wvp = tc.alloc_tile_pool(name="wvp", bufs=2)
    for half in range(2):
        wvt = wvp.tile([128, DC, 256], f32r, tag="wvh")
        nc.sync.dma_start(
            wvt[:],
            wv_d.ap()[:, half * 256:(half + 1) * 256]
            .rearrange("(c p) e -> p c e", p=128).bitcast(f32r))
        for jc in range(NJC):
            pv = ps_acc.tile([128, 256], f32, tag="acc")
            for dc in range(DC):
                mm(pv[:], xTc(dc, jc * 128, (jc + 1) * 128), wvt[:, dc, :],
                   start=(dc == 0), stop=(dc == DC - 1))
            nc.vector.tensor_copy(vv[:, jc, half * 256:(half + 1) * 256], pv[:])

    wvp.release()
    xl.release()
    p0a.release()
    ps_acc.release()

    if stop_after == "kv":
        if lp is not None:
            lp.__exit__(None, None, None)
            dtile = pers.tile([128, 128], f32, tag="dtile")
            nc.vector.memset(dtile[:], 0.0)
            nc.sync.dma_start(dummy_d.ap(), dtile[0:1, :])
        wp.release()
        p0b.release()
        proj.release()
        ps_ot.release()
        ps_s.release()
        pers.release()
        return

    # ---------------- P2: attention, q projection interleaved ----------
    attn = tc.alloc_tile_pool(name="attn", bufs=1, side="right")
    sm = tc.alloc_tile_pool(name="sm", bufs=2, side="right")
    ps_wtp = tc.alloc_tile_pool(name="ps_wtp", bufs=2, space="PSUM",
                                side="right")

    oT = attn.tile([128, H, TLOC], f32r, tag="oT")
    wT = attn.tile([128, NJC, TLOC], f32r, tag="wT")
    # zero the out-of-band blocks of wT once (via f32 scratch; gpsimd
    # memset cannot encode f32r)
    zblk = attn.tile([128, 384], f32, tag="zblk")
    nc.gpsimd.memset(zblk[:], 0.0)
    for jc, (a, b) in enumerate(
            [(128, 512), (256, 512), (384, 512), (None, None),
             (None, None), (0, 128), (0, 256), (0, 384)]):
        if a is not None:
            nc.vector.tensor_copy(wT[:, jc, a:b], zblk[:, 0:b - a])

    def qproj(h):
        """q projection for head h (runs one head ahead of scores)."""
        wqh = wp.tile([128, DC, 128], f32r, tag="wlhs", name=f"wqh{h}")
        nc.sync.dma_start(
            wqh[:],
            wq_d.ap()[:, h * 128:(h + 1) * 128]
            .rearrange("(c p) e -> p c e", p=128).bitcast(f32r))
        pq = ps_ot.tile([128, TLOC], f32, tag="ot", name=f"pq{h}")
        for dc in range(DC):
            mm(pq[:], wqh[:, dc, :], xTb[:, dc, :],
               start=(dc == 0), stop=(dc == DC - 1))
        nc.scalar.mul(qT[:, h, :], pq[:], SCALE)

    def stage_a(h):
        """scores + softmax for head h (qT already resident)."""
        g = h // (H // KVH)
        wt_list = []
        lsum4 = sm.tile([128, NT], f32, tag="l4", name=f"l4_{h}", bufs=2)
        r4 = sm.tile([128, NT], f32, tag="r4", name=f"r4_{h}", bufs=2)
        for t in range(NT):
            ps = ps_s.tile([128, 1024], f32, tag="score", name=f"s{h}_{t}")
            mm(ps[:, 0:512], qT[:, h, t * 128:(t + 1) * 128],
               kT[:, g, t * 128:t * 128 + 512], start=True, stop=True)
            mm(ps[:, 512:BAND], qT[:, h, t * 128:(t + 1) * 128],
               kT[:, g, t * 128 + 512:t * 128 + BAND], start=True, stop=True)
            w = sm.tile([128, BAND], f32r, tag="w", name=f"w{h}_{t}", bufs=10)
            nc.scalar.activation(w[:], ps[:, 0:BAND],
                                 mybir.ActivationFunctionType.Exp)
            # multiplicative mask + row-sum in one DVE pass (in place)
            nc.vector.scalar_tensor_tensor(
                w[:], w[:], 1.0, mask_s[:, t, :],
                op0=mybir.AluOpType.mult, op1=mybir.AluOpType.mult,
                accum_out=lsum4[:, t:t + 1])
            wt_list.append(w)
        nc.vector.reciprocal(r4[:], lsum4[:])
        for t in range(NT):
            if t < 2:
                nc.vector.tensor_scalar_mul(wt_list[t][:], wt_list[t][:],
                                            r4[:, t:t + 1])
            else:
                nc.scalar.mul(wt_list[t][:], wt_list[t][:], r4[:, t:t + 1])
        return wt_list

    def stage_b(h, wt_list):
        """transpose + PV + oT for head h."""
        g = h // (H // KVH)
        for jc in range(NJC):
            t_lo = max(0, jc - 4)
            t_hi = min(NT - 1, jc)
            pt = ps_wtp.tile([128, 512], f32r, tag="wtp",
                             name=f"pt{h}_{jc}")
            for t in range(t_lo, t_hi + 1):
                co = jc - t  # w column block
                nc.tensor.transpose(
                    pt[:, t * 128:(t + 1) * 128],
                    wt_list[t][:, co * 128:(co + 1) * 128],
                    identr[:])
            if jc % 2 == 0:
                nc.vector.tensor_copy(
                    wT[:, jc, t_lo * 128:(t_hi + 1) * 128],
                    pt[:, t_lo * 128:(t_hi + 1) * 128])
            else:
                nc.scalar.copy(
                    wT[:, jc, t_lo * 128:(t_hi + 1) * 128],
                    pt[:, t_lo * 128:(t_hi + 1) * 128])

        po = ps_ot.tile([128, TLOC], f32, tag="ot", name=f"po{h}")
        for jc in range(NJC):
            mm(po[:], vv[:, jc, g * 128:(g + 1) * 128], wT[:, jc, :],
               start=(jc == 0), stop=(jc == NJC - 1))
        nc.vector.tensor_copy(oT[:, h, :], po[:])

    # 2-stage software pipeline: head h's scores/softmax are emitted
    # before head h-1's transposes/PV so the PE queue always has work
    # while DVE/ACT/Pool run the softmax chain.
    prev = None
    qproj(0)
    for h in range(H):
        cur = stage_a(h)
        if h + 1 < H:
            qproj(h + 1)
        if prev is not None:
            stage_b(h - 1, prev)
        prev = cur
    stage_b(H - 1, prev)

    sm.release()
    ps_wtp.release()
    wp.release()
    p0b.release()
    proj.release()

    if stop_after == "attn":
        if lp is not None:
            lp.__exit__(None, None, None)
            dtile = pers.tile([128, 128], f32, tag="dtile")
            nc.vector.memset(dtile[:], 0.0)
            nc.sync.dma_start(dummy_d.ap(), dtile[0:1, :])
        attn.release()
        ps_ot.release()
        ps_s.release()
        pers.release()
        return

    # ---------------- P3: output projection ----------------
    wop = tc.alloc_tile_pool(name="wo_pool", bufs=3, side="right")
    ps_acc2 = tc.alloc_tile_pool(name="ps_acc2", bufs=2, space="PSUM",
                                 side="right")
    for dblk in range(4):
        woc = wop.tile([128, H, 512], f32r, tag="wo")
        nc.sync.dma_start(
            woc[:],
            wo_d.ap()[:, dblk * 512:(dblk + 1) * 512]
            .rearrange("(h p) e -> p h e", p=128).bitcast(f32r))
        for t in range(NT):
            py = ps_acc2.tile([128, 512], f32, tag="acc")
            for h in range(H):
                mm(py[:], oT[:, h, t * 128:(t + 1) * 128], woc[:, h, :],
                   start=(h == 0), stop=(h == H - 1))
            ych = attn.tile([128, 512], f32, tag="ych", bufs=2)
            nc.vector.tensor_copy(ych[:], py[:])
            nc.scalar.dma_start(
                y_d.ap()[t * 128:(t + 1) * 128,
                         dblk * 512:(dblk + 1) * 512],
                ych[:])

    ps_acc2.release()
    wop.release()
    attn.release()

    if lp is not None:
        lp.__exit__(None, None, None)
        dtile = pers.tile([128, 128], f32, tag="dtile")
        nc.vector.memset(dtile[:], 0.0)
        nc.sync.dma_start(dummy_d.ap(), dtile[0:1, :])

    ps_ot.release()
    ps_s.release()
    pers.release()


def build_nc(loop_n=None, stop_after=None):
    key = ("nc", loop_n, stop_after)
    if key in _CACHE:
        return _CACHE[key]
    import concourse.bacc as bacc
    import concourse.mybir as mybir
    import concourse.tile as tile
    from concourse.masks import make_identity

    nc = bacc.Bacc("TRN2", target_bir_lowering=False, debug=False,
                   num_devices=N_CORES)
    with tile.TileContext(nc) as tc:
        _emit(nc, tc, tile, mybir, make_identity, loop_n=loop_n,
          stop_after=stop_after)
    nc.compile()
    _CACHE[key] = nc
    return nc


def make_inputs_for_core(c, xf, Wq, Wk, Wv, Wo):
    """xf: [T, D] float32 (already squeezed)."""
    if c == 0:
        x_c = np.concatenate(
            [np.zeros((WINDOW, D), np.float32), xf[:TLOC]], axis=0)
    else:
        x_c = xf[TLOC * c - WINDOW: TLOC * c + TLOC]
    x_c = np.ascontiguousarray(x_c, dtype=np.float32)

    jj = np.arange(BAND)[None, None, :]
    p = np.arange(128)[None, :, None]
    t = np.arange(NT)[:, None, None]
    allowed = (jj >= p) & (jj <= p + WINDOW)
    if c == 0:
        allowed = allowed & (128 * t + jj >= WINDOW)
    allowed = np.broadcast_to(allowed, (NT, 128, BAND))
    mask = np.where(allowed, np.float32(1.0),
                    np.float32(0.0)).astype(np.float32)

    return {
        "x": x_c,
        "wq": np.ascontiguousarray(Wq, np.float32),
        "wk": np.ascontiguousarray(Wk, np.float32),
        "wv": np.ascontiguousarray(Wv, np.float32),
        "wo": np.ascontiguousarray(Wo, np.float32),
        "mask": mask,
    }


def kernel(x, Wq, Wk, Wv, Wo):
    from concourse.bass_utils import run_bass_kernel_spmd

    nc = build_nc()
    xf = np.asarray(x, np.float32).reshape(T, D)
    in_maps = [make_inputs_for_core(c, xf, Wq, Wk, Wv, Wo)
               for c in range(N_CORES)]
    res = run_bass_kernel_spmd(nc, in_maps, core_ids=list(range(N_CORES)))
    y = np.concatenate([res.results[c]["y"] for c in range(N_CORES)], axis=0)
    return y.reshape(1, T, D)
